# revision 1
# baseline (speedup 1.0000x reference)
"""Connectome kernel (segment-mean -> Pearson Gram) for 8 TRN2 NeuronCores.

Strategy (pure data parallel, 2 samples per core):
  - x (16,1,200,144,320) fp32 -> per-core slice (400 rows, 46080 pixels).
  - Stream x naturally (rows on partitions) via the ACT HWDGE ring,
    cast fp32->fp16 on DVE, transpose pixel-major via the DMA xbar
    (ucode DMA_TRANSPOSE, fp16, SP HWDGE ring).
  - Segment-sum as matmul: roi.T[r, row] += onehot[v,r].T @ xT[v, row],
    fp16 operands, fp32 PSUM accumulation over 360 pixel chunks.
    onehot built per chunk on DVE from the parcellation (is_equal vs iota).
  - Epilogue per core (fp32): scale by 1/count, demean over t, normalize,
    transpose (PE), Gram matmul, write (2,200,200) conn to HBM.
  - Host: concat cores, extract upper triangle -> (16, 19900).
"""
import sys

sys.path.insert(0, "/opt/trn_rl_repo")

import numpy as np

import concourse.bass as bass
import concourse.tile as tile
from concourse import bacc, mybir
from concourse.bass_utils import run_bass_kernel_spmd

F32 = mybir.dt.float32
F16 = mybir.dt.float16

N, T, H, W = 16, 200, 144, 320
V = H * W                      # 46080
R = 200                        # ROIs
NCORES = 8
SPB = N // NCORES              # samples per core = 2
ROWS = SPB * T                 # 400
NCHUNK = V // 128              # 360
SPAN = 2048                    # pixels per load span
EPS = 1e-8

_cached = {}


def _build_program():
    nc = bacc.Bacc("TRN2", target_bir_lowering=False, debug=False)

    x_d = nc.declare_dram_parameter("x", [ROWS, V], F32, isOutput=False)
    parc_d = nc.declare_dram_parameter("parcv", [128, NCHUNK], F32, isOutput=False)
    iota_d = nc.declare_dram_parameter("iota", [128, R], F32, isOutput=False)
    invca_d = nc.declare_dram_parameter("invca", [128, 1], F32, isOutput=False)
    invcb_d = nc.declare_dram_parameter("invcb", [72, 1], F32, isOutput=False)
    i128_d = nc.declare_dram_parameter("i128", [128, 128], F32, isOutput=False)
    i72_d = nc.declare_dram_parameter("i72", [72, 72], F32, isOutput=False)
    out_d = nc.declare_dram_parameter("conn", [SPB, R, R], F32, isOutput=True)

    spans = []
    v0 = 0
    while v0 < V:
        f = min(SPAN, V - v0)
        spans.append((v0, f))
        v0 += f

    with tile.TileContext(nc) as tc:
        with tc.tile_pool(name="consts", bufs=1) as consts, \
             tc.tile_pool(name="loads", bufs=2) as loads, \
             tc.tile_pool(name="f16s", bufs=2) as f16s, \
             tc.tile_pool(name="xts", bufs=2) as xts, \
             tc.tile_pool(name="ohp", bufs=4) as ohp, \
             tc.tile_pool(name="epi", bufs=1) as epi, \
             tc.tile_pool(name="psum", bufs=1, space="PSUM") as psum:

            parc_s = consts.tile([128, NCHUNK], F32)
            iota_s = consts.tile([128, R], F32)
            invca_s = consts.tile([128, 1], F32)
            invcb_s = consts.tile([72, 1], F32)
            i128_s = consts.tile([128, 128], F32)
            i72_s = consts.tile([72, 72], F32)
            nc.sync.dma_start(parc_s[:], parc_d[:])
            nc.sync.dma_start(iota_s[:], iota_d[:])
            nc.sync.dma_start(invca_s[:], invca_d[:])
            nc.sync.dma_start(invcb_s[:], invcb_d[:])
            nc.sync.dma_start(i128_s[:], i128_d[:])
            nc.sync.dma_start(i72_s[:], i72_d[:])

            acc_a = psum.tile([128, ROWS], F32, tag="acc_a", bufs=1)
            acc_b = psum.tile([72, ROWS], F32, tag="acc_b", bufs=1)

            with nc.named_scope("main"):
                cc = 0
                for (v0, f) in spans:
                    nblk = f // 128
                    # fp32 natural loads (ACT HWDGE ring), rows on partitions
                    lds = []
                    for rb in range(3):
                        ld = loads.tile([128, f], F32, tag=f"ld{rb}", bufs=2,
                                        name=f"ld{rb}_{v0}")
                        nc.scalar.dma_start(ld[:], x_d[rb * 128:(rb + 1) * 128,
                                                       v0:v0 + f])
                        lds.append(ld)
                    ldt = loads.tile([16, f], F32, tag="ldt", bufs=2,
                                     name=f"ldt_{v0}")
                    nc.scalar.dma_start(ldt[:], x_d[384:400, v0:v0 + f])

                    # cast to fp16 (DVE)
                    f16n = []
                    for rb in range(3):
                        ft = f16s.tile([128, f], F16, tag=f"f16_{rb}", bufs=2,
                                       name=f"f16{rb}_{v0}")
                        nc.vector.tensor_copy(ft[:], lds[rb][:])
                        f16n.append(ft)
                    ftt = f16s.tile([16, f], F16, tag="f16_t", bufs=2,
                                    name=f"f16t_{v0}")
                    nc.vector.tensor_copy(ftt[:], ldt[:])

                    # xbar transposes (SP HWDGE ring) into (v, blk, row)
                    xt = xts.tile([128, nblk, ROWS], F16, tag="xt", bufs=2,
                                  name=f"xt_{v0}")
                    for rb in range(3):
                        nc.sync.dma_start(xt[:, :, rb * 128:(rb + 1) * 128],
                                          f16n[rb][:], transpose=True)
                    nc.sync.dma_start(xt[:, :, 384:400], ftt[:], transpose=True)

                    # per 128-pixel chunk: onehot + 2 accumulating matmuls
                    for c in range(nblk):
                        oh = ohp.tile([128, R], F16, tag="oh", bufs=4,
                                      name=f"oh_{cc}")
                        nc.vector.tensor_scalar(oh[:], iota_s[:],
                                                parc_s[:, cc:cc + 1], None,
                                                op0=mybir.AluOpType.is_equal)
                        nc.tensor.matmul(acc_a[:], oh[:, 0:128], xt[:, c, :],
                                         start=(cc == 0), stop=(cc == NCHUNK - 1))
                        nc.tensor.matmul(acc_b[:], oh[:, 128:200], xt[:, c, :],
                                         start=(cc == 0), stop=(cc == NCHUNK - 1))
                        cc += 1

            with nc.named_scope("epilogue"):
                # roi sums -> sbuf, scale by 1/count
                roi_a = epi.tile([128, ROWS], F32)
                roi_b = epi.tile([72, ROWS], F32)
                nc.vector.tensor_copy(roi_a[:], acc_a[:])
                nc.vector.tensor_copy(roi_b[:], acc_b[:])
                nc.vector.tensor_scalar_mul(roi_a[:], roi_a[:], invca_s[:])
                nc.vector.tensor_scalar_mul(roi_b[:], roi_b[:], invcb_s[:])

                for s in range(SPB):
                    sl = bass.ts(s, T)
                    roiN = {}
                    for blk, rt, P, invc in (("a", roi_a, 128, invca_s),
                                             ("b", roi_b, 72, invcb_s)):
                        mean = epi.tile([P, 1], F32, name=f"mean_{blk}{s}",
                                        tag=f"mean_{blk}")
                        nc.vector.tensor_reduce(mean[:], rt[:, sl],
                                                axis=mybir.AxisListType.X,
                                                op=mybir.AluOpType.add)
                        nc.vector.tensor_scalar_mul(mean[:], mean[:], 1.0 / T)
                        rc = epi.tile([P, T], F32, name=f"rc_{blk}{s}",
                                      tag=f"rc_{blk}")
                        nc.vector.tensor_scalar(rc[:], rt[:, sl], mean[:], None,
                                                op0=mybir.AluOpType.subtract)
                        sq = epi.tile([P, T], F32, name=f"sq_{blk}{s}",
                                      tag=f"sq_{blk}")
                        ss = epi.tile([P, 1], F32, name=f"ss_{blk}{s}",
                                      tag=f"ss_{blk}")
                        nc.vector.scalar_tensor_tensor(
                            sq[:], rc[:], 1.0, rc[:],
                            op0=mybir.AluOpType.mult, op1=mybir.AluOpType.mult,
                            accum_out=ss[:])
                        nc.scalar.sqrt(ss[:], ss[:])
                        nc.vector.tensor_scalar_add(ss[:], ss[:], EPS)
                        nc.vector.reciprocal(ss[:], ss[:])
                        rn = epi.tile([P, T], F32, name=f"rn_{blk}{s}",
                                      tag=f"rn_{blk}")
                        nc.vector.tensor_scalar_mul(rn[:], rc[:], ss[:])
                        roiN[blk] = rn

                    # transpose roiN -> (t, r) on PE
                    trA = psum.tile([128, R], F32, tag="trA", bufs=1,
                                    name=f"trA_{s}")
                    trB = psum.tile([72, R], F32, tag="trB", bufs=1,
                                    name=f"trB_{s}")
                    nc.tensor.transpose(trA[:, 0:128], roiN["a"][:, 0:128], i128_s[:])
                    nc.tensor.transpose(trA[:, 128:200], roiN["b"][:, 0:128], i72_s[:])
                    nc.tensor.transpose(trB[:, 0:128], roiN["a"][:, 128:200], i128_s[:])
                    nc.tensor.transpose(trB[:, 128:200], roiN["b"][:, 128:200], i72_s[:])
                    trA_sb = epi.tile([128, R], F32, name=f"trAs_{s}", tag="trAs")
                    trB_sb = epi.tile([72, R], F32, name=f"trBs_{s}", tag="trBs")
                    nc.vector.tensor_copy(trA_sb[:], trA[:])
                    nc.vector.tensor_copy(trB_sb[:], trB[:])

                    # Gram: conn = roiN_t.T @ roiN_t  (contraction over t)
                    cA = psum.tile([128, R], F32, tag="cA", bufs=1, name=f"cA_{s}")
                    cB = psum.tile([72, R], F32, tag="cB", bufs=1, name=f"cB_{s}")
                    nc.tensor.matmul(cA[:], trA_sb[:, 0:128], trA_sb[:],
                                     start=True, stop=False)
                    nc.tensor.matmul(cA[:], trB_sb[:, 0:128], trB_sb[:],
                                     start=False, stop=True)
                    nc.tensor.matmul(cB[:], trA_sb[:, 128:200], trA_sb[:],
                                     start=True, stop=False)
                    nc.tensor.matmul(cB[:], trB_sb[:, 128:200], trB_sb[:],
                                     start=False, stop=True)
                    cA_sb = epi.tile([128, R], F32, name=f"cAs_{s}", tag="cAs")
                    cB_sb = epi.tile([72, R], F32, name=f"cBs_{s}", tag="cBs")
                    nc.vector.tensor_copy(cA_sb[:], cA[:])
                    nc.vector.tensor_copy(cB_sb[:], cB[:])
                    nc.sync.dma_start(out_d[s, 0:128, :], cA_sb[:])
                    nc.sync.dma_start(out_d[s, 128:200, :], cB_sb[:])

    nc.compile()
    return nc


def _get_program():
    if "nc" not in _cached:
        _cached["nc"] = _build_program()
    return _cached["nc"]


def kernel(x, parc, mask):
    x = np.asarray(x, dtype=np.float32)
    parc = np.asarray(parc)
    mask = np.asarray(mask)

    # host-side prep (tiny): effective parcellation and inverse counts
    parc_eff = np.where(mask, parc, 0).reshape(V).astype(np.int64)
    counts = np.bincount(parc_eff, minlength=R + 1).astype(np.float32)
    inv = np.float32(1.0) / counts[1:]                      # (200,)
    lab = (parc_eff - 1).astype(np.float32)                 # -1 for background
    parcv = lab.reshape(NCHUNK, 128).T.copy()               # (128, 360)
    iota = np.broadcast_to(np.arange(R, dtype=np.float32), (128, R)).copy()
    invca = inv[0:128].reshape(128, 1).copy()
    invcb = inv[128:200].reshape(72, 1).copy()
    i128 = np.eye(128, dtype=np.float32)
    i72 = np.eye(72, dtype=np.float32)

    xr = x.reshape(N, T, V)
    in_maps = []
    for c in range(NCORES):
        in_maps.append({
            "x": np.ascontiguousarray(
                xr[c * SPB:(c + 1) * SPB].reshape(ROWS, V)),
            "parcv": parcv, "iota": iota,
            "invca": invca, "invcb": invcb,
            "i128": i128, "i72": i72,
        })

    nc = _get_program()
    res = run_bass_kernel_spmd(nc, in_maps, core_ids=list(range(NCORES)))
    conn = np.concatenate([r["conn"] for r in res.results], axis=0)  # (16,200,200)
    row, col = np.triu_indices(R, k=1)
    return np.ascontiguousarray(conn[:, row, col]).astype(np.float32)
